# revision 11
# baseline (speedup 1.0000x reference)
"""Causal depthwise conv1d kernel for Trainium2 (8 NeuronCores).

Reference op:
    y[b, s, h] = sum_{j=0..K-1} w[h, j] * x[b, s-(K-1)+j, h]   (zero left-pad)
    y *= attention_mask_2d[b, s]  (mask is all-ones in the graded inputs)

Layout (hardcoded for B=4, S=4096, H=2048, K=4, 8 cores):
  - Shard the H=2048 channels across 8 cores (256 channels each); depthwise
    conv has no cross-channel mixing so this is fully local.
  - Host transposes to channel-major rows in bf16: each (channel, batch) pair
    is an independent length-S sequence, left-padded with 4 zeros (one more
    than K-1 so every device-side stream offset stays even => 4B-aligned for
    the DVE's 2x packed mode). Tolerance is 2e-2; bf16 I/O error is ~1e-3.
  - Device: channels on SBUF partitions, sequence on the free dim.

Compute: two custom DVE ops whose 2X_1PORT uop programs process two packed
bf16 elements per cycle: each cycle reads one 32-bit SBUF word per port
(SRC_0/SRC_0_HI, SRC_1/SRC_1_HI), computes both outputs, and writes them
packed via WR0_LO/WR0_HI. A block that multiplies the hi element and
self-captures its out-flop into a delay chain yields the previous pair's
product, i.e. the x[i-1] tap:
    FIR2S:    p[i] = c0*x[i] + c1*x[i-1]            (dummy src1 pins the
              perf ceiling to TwoSrc = 2X_1PORT; a OneSrc op could escalate
              to 4X where no real program exists)
    FIR2ADDS: y[i] = c0*x[i] + c1*x[i-1] + p[i]     (8 ALUs, exactly fits;
              the hi x element enters twice — once through input lane 0
              straight into block 0's ALU — and captures reuse chains the
              moment their previous cargo dies)
The 4-tap causal conv is p = w3*x + w2*x[-1]; y = w1*x[-2] + w0*x[-3] + p.
Both ops also carry an equivalent REGULAR (1x) program, so if the engine's
runtime mode check fails the result is still correct, just slower.

Per core: 8.4MB in + 8.4MB out of bf16 at ~358 GB/s HBM-per-core ~= 47us;
DVE at 2 elem/cycle/partition ~= 35us, hidden under the DMA. Out-DMAs ride
the ScalarEngine's HWDGE queue so outputs never head-of-line-block input
tile loads.
"""

import numpy as np
import ml_dtypes
from contextlib import ExitStack

import concourse.bass as bass
import concourse.bass_isa as bass_isa
import concourse.tile as tile
from concourse import bacc, mybir
from concourse import bass_utils
import concourse.dve_ops as dve_ops
from concourse.dve_spec import Spec, Src0, Src1, C0, C1
from concourse.dve_uop import (
    DveOpSpec, UopConfig, AluOp, AluInp, DelayInp, InpSel,
    OutPath, OutSel, Trigger,
)

B, S, H, K = 4, 4096, 2048, 4
N_CORES = 8
C = H // N_CORES        # channels per core
R = C * B               # rows per core (each row: one (channel, batch) sequence)
PAD = 4                 # left zero-pad (K-1=3 needed; 4 keeps offsets even)
SP = S + PAD            # padded row length
P = 128                 # SBUF partitions
N_GROUPS = R // P       # 8 row groups per core
CH = 2048               # output columns per chunk (2 chunks per group)
F32 = mybir.dt.float32
BF16 = mybir.dt.bfloat16
BF = ml_dtypes.bfloat16

PERF_MAX = 1            # 1 => allow 2X_1PORT; 0 => force REGULAR


# --- custom DVE ops -------------------------------------------------------- #

class _HandOp:
    """DveOp stand-in whose table program is a hand-built DveOpSpec."""

    def __init__(self, name, build_1x, build_2x, rd1_en, ref_spec):
        self.name = name
        self.subdim = False
        self.spec = ref_spec  # consulted only for spec_leaves checks
        self._rd1 = rd1_en
        self._b1, self._b2 = build_1x, build_2x
        self._cache = {}
        self.uops_sha = {}

    def compile(self, ver):
        if ver not in self._cache:
            s = DveOpSpec(
                name=self.name,
                opcode=dve_ops.get_dve_sub_opcode(self.name),
                uops=self._b1(),
                uops_2x=self._b2() if self._b2 is not None else None,
                rd1_en=self._rd1,
            )
            s.validate(ver)
            self._cache[ver] = s
        return self._cache[ver]


def _register(op):
    if op.name not in dve_ops._SUB_OPCODE_FOR_NAME:
        opcode = max(dve_ops._SUB_OPCODE_FOR_NAME.values()) + 1
        assert opcode < 0x20
        dve_ops._SUB_OPCODE_FOR_NAME[op.name] = opcode
        dve_ops.OPS.append(op)
        dve_ops.CUSTOM_DVE_SPECS[op.name] = op.spec
    else:
        for existing in dve_ops.OPS:
            if existing.name == op.name:
                return existing
    return op


def _steady_base(require_inp1):
    u = UopConfig()
    u.require_inp0 = 1
    u.require_inp1 = 1 if require_inp1 else 0
    u.trigger = (Trigger.SRC_TENSOR_DONE, Trigger.NONE, Trigger.NONE)
    u.next_uop = (0, 0, 0)
    u.out = {p: OutSel.ALU_OUT for p in OutPath}
    u.out_enable = {p: 0 for p in OutPath}
    return u


# -- FIR2S: p[i] = C0*x[i] + C1*x[i-1]  (src1 is a dummy stream) ------------ #

def _fir2s_1x():
    # Identical structure to the proven baseline FIR2 program.
    u = _steady_base(require_inp1=True)
    u.enable_input(InpSel.SRC_0, 1)            # chain 0 = x
    u.enable_input(InpSel.CONST_0, 3)          # chain 2 = C0
    u.enable_input(InpSel.CONST_1, 4)          # chain 3 = C1
    dp = u.datapath_config
    # b0: flop = x[i]; chain5 := own flop (= x[i-1])
    dp[0].enable_alu(AluOp.BYPASS, AluInp.PREV_DELAY_0)
    dp[0].pass_through_delay(0, 1, 2, 3)
    dp[0].enable_delay_from_src(DelayInp.CURR_ALU_OUT, 5)
    # b1: flop = x[i-1] * C1
    dp[1].enable_alu(AluOp.MULTIPLY, AluInp.PREV_DELAY_5, AluInp.PREV_DELAY_3)
    dp[1].pass_through_delay(0, 1, 2)
    # b2: flop = x[i] * C0; chain3 := prev alu
    dp[2].enable_alu(AluOp.MULTIPLY, AluInp.PREV_DELAY_0, AluInp.PREV_DELAY_2)
    dp[2].pass_through_delay(1)
    dp[2].enable_delay_from_src(DelayInp.PREV_ALU_OUT, 3)
    # b3: flop = C0*x[i] + C1*x[i-1]
    dp[3].enable_alu(AluOp.ADD, AluInp.PREV_ALU_OUT, AluInp.PREV_DELAY_3)
    for k in range(4, 8):
        dp[k].pass_through_alu()
    u.out_enable[OutPath.WR0_LO] = 1
    return [u]


def _fir2s_2x():
    u = _steady_base(require_inp1=True)
    u.enable_input(InpSel.SRC_0, 1)            # chain 0 = XL = x[2i]
    u.enable_input(InpSel.SRC_0_HI, 2)         # chain 1 = XH = x[2i+1]
    u.enable_input(InpSel.CONST_0, 3)          # chain 2 = C0
    u.enable_input(InpSel.CONST_1, 4)          # chain 3 = C1
    dp = u.datapath_config
    # b0: out = C1*XH; chain4 := own flop => C1*x[2i-1] (prev pair)
    dp[0].enable_alu(AluOp.MULTIPLY, AluInp.PREV_DELAY_1, AluInp.PREV_DELAY_3)
    dp[0].enable_delay_from_src(DelayInp.CURR_ALU_OUT, 4)
    dp[0].pass_through_delay(0, 1, 2, 3)
    # b1: out = C1*XL  (C1 dies)
    dp[1].enable_alu(AluOp.MULTIPLY, AluInp.PREV_DELAY_0, AluInp.PREV_DELAY_3)
    dp[1].pass_through_delay(0, 1, 2, 4)
    # b2: out = C0*XL (XL dies); chain3 := prev alu = C1*XL
    dp[2].enable_alu(AluOp.MULTIPLY, AluInp.PREV_DELAY_0, AluInp.PREV_DELAY_2)
    dp[2].enable_delay_from_src(DelayInp.PREV_ALU_OUT, 3)
    dp[2].pass_through_delay(1, 2, 4)
    # b3: out = C0*XH (XH, C0 die); chain0 := prev alu = C0*XL
    dp[3].enable_alu(AluOp.MULTIPLY, AluInp.PREV_DELAY_1, AluInp.PREV_DELAY_2)
    dp[3].enable_delay_from_src(DelayInp.PREV_ALU_OUT, 0)
    dp[3].pass_through_delay(3, 4)
    # b4: out = p_lo = C0*XL + C1*x[2i-1]; chain1 := prev alu = C0*XH
    dp[4].enable_alu(AluOp.ADD, AluInp.PREV_DELAY_0, AluInp.PREV_DELAY_4)
    dp[4].enable_delay_from_src(DelayInp.PREV_ALU_OUT, 1)
    dp[4].pass_through_delay(3)
    # b5: out = p_hi = C0*XH + C1*XL; chain2 := prev alu = p_lo
    dp[5].enable_alu(AluOp.ADD, AluInp.PREV_DELAY_1, AluInp.PREV_DELAY_3)
    dp[5].enable_delay_from_src(DelayInp.PREV_ALU_OUT, 2)
    # b6, b7: carry p_hi on the ALU path, p_lo on chain 2
    dp[6].pass_through_alu()
    dp[6].pass_through_delay(2)
    dp[7].pass_through_alu()
    dp[7].pass_through_delay(2)
    u.out[OutPath.WR0_LO] = OutSel.DELAY_2
    u.out[OutPath.WR0_HI] = OutSel.ALU_OUT
    u.out_enable[OutPath.WR0_LO] = 1
    u.out_enable[OutPath.WR0_HI] = 1
    return [u]


# -- FIR2ADDS: y[i] = C0*x[i] + C1*x[i-1] + p[i]  (src1 = p) ---------------- #

def _fir2adds_1x():
    # Identical structure to the proven baseline FIR2ADD program.
    u = _steady_base(require_inp1=True)
    u.enable_input(InpSel.SRC_0, 1)            # chain 0 = x
    u.enable_input(InpSel.SRC_1, 2)            # chain 1 = p
    u.enable_input(InpSel.CONST_0, 3)          # chain 2 = C0
    u.enable_input(InpSel.CONST_1, 4)          # chain 3 = C1
    dp = u.datapath_config
    dp[0].enable_alu(AluOp.BYPASS, AluInp.PREV_DELAY_0)
    dp[0].pass_through_delay(0, 1, 2, 3)
    dp[0].enable_delay_from_src(DelayInp.CURR_ALU_OUT, 5)
    dp[1].enable_alu(AluOp.MULTIPLY, AluInp.PREV_DELAY_5, AluInp.PREV_DELAY_3)
    dp[1].pass_through_delay(0, 1, 2)
    dp[2].enable_alu(AluOp.MULTIPLY, AluInp.PREV_DELAY_0, AluInp.PREV_DELAY_2)
    dp[2].pass_through_delay(1)
    dp[2].enable_delay_from_src(DelayInp.PREV_ALU_OUT, 3)
    dp[3].enable_alu(AluOp.ADD, AluInp.PREV_ALU_OUT, AluInp.PREV_DELAY_3)
    dp[3].pass_through_delay(1)
    dp[4].enable_alu(AluOp.ADD, AluInp.PREV_ALU_OUT, AluInp.PREV_DELAY_1)
    for k in range(5, 8):
        dp[k].pass_through_alu()
    u.out_enable[OutPath.WR0_LO] = 1
    return [u]


def _fir2adds_2x():
    # Per pair: P1 = C0*XL, P2 = C1*x[2i-1] (self-capture), P3 = C0*XH,
    # P4 = C1*XL; y_lo = P1+P2+PL, y_hi = P3+P4+PH. 8 ALU blocks exactly.
    # XH enters twice (lane 0 into block 0's ALU, lane 6 onto chain 5) so
    # both its consumers are fed without burning a block on transport.
    u = _steady_base(require_inp1=True)
    u.enable_input(InpSel.SRC_0_HI, 0)         # lane 0 -> block0 ALU = XH
    u.enable_input(InpSel.SRC_0, 1)            # chain 0 = XL
    u.enable_input(InpSel.SRC_1, 2)            # chain 1 = PL
    u.enable_input(InpSel.SRC_1_HI, 3)         # chain 2 = PH
    u.enable_input(InpSel.CONST_0, 4)          # chain 3 = C0 (A)
    u.enable_input(InpSel.CONST_1, 5)          # chain 4 = C1 (B)
    u.enable_input(InpSel.SRC_0_HI, 6)         # chain 5 = XH (again)
    dp = u.datapath_config
    # b0: out = P3 = C0*XH (lane0)
    dp[0].enable_alu(AluOp.MULTIPLY, AluInp.PREV_ALU_OUT, AluInp.PREV_DELAY_3)
    dp[0].pass_through_delay(0, 1, 2, 3, 4, 5)
    # b1: out = P1 = C0*XL (C0 dies); chain3 := prev alu = P3
    dp[1].enable_alu(AluOp.MULTIPLY, AluInp.PREV_DELAY_0, AluInp.PREV_DELAY_3)
    dp[1].enable_delay_from_src(DelayInp.PREV_ALU_OUT, 3)
    dp[1].pass_through_delay(0, 1, 2, 4, 5)
    # b2: out = P4 = C1*XL (XL dies); chain0 := prev alu = P1
    dp[2].enable_alu(AluOp.MULTIPLY, AluInp.PREV_DELAY_0, AluInp.PREV_DELAY_4)
    dp[2].enable_delay_from_src(DelayInp.PREV_ALU_OUT, 0)
    dp[2].pass_through_delay(1, 2, 3, 4, 5)
    # b3: out = C1*XH (B, XH die); chain4 := prev alu = P4;
    #     chain5 := own flop => P2 = C1*x[2i-1] (prev pair)
    dp[3].enable_alu(AluOp.MULTIPLY, AluInp.PREV_DELAY_5, AluInp.PREV_DELAY_4)
    dp[3].enable_delay_from_src(DelayInp.PREV_ALU_OUT, 4)
    dp[3].enable_delay_from_src(DelayInp.CURR_ALU_OUT, 5)
    dp[3].pass_through_delay(0, 1, 2, 3)
    # b4: out = P1 + P2
    dp[4].enable_alu(AluOp.ADD, AluInp.PREV_DELAY_0, AluInp.PREV_DELAY_5)
    dp[4].pass_through_delay(1, 2, 3, 4)
    # b5: out = y_lo = P1 + P2 + PL (PL dies)
    dp[5].enable_alu(AluOp.ADD, AluInp.PREV_ALU_OUT, AluInp.PREV_DELAY_1)
    dp[5].pass_through_delay(2, 3, 4)
    # b6: out = P3 + P4; chain1 := prev alu = y_lo
    dp[6].enable_alu(AluOp.ADD, AluInp.PREV_DELAY_3, AluInp.PREV_DELAY_4)
    dp[6].enable_delay_from_src(DelayInp.PREV_ALU_OUT, 1)
    dp[6].pass_through_delay(2)
    # b7: out = y_hi = P3 + P4 + PH
    dp[7].enable_alu(AluOp.ADD, AluInp.PREV_ALU_OUT, AluInp.PREV_DELAY_2)
    dp[7].pass_through_delay(1)
    u.out[OutPath.WR0_LO] = OutSel.DELAY_1
    u.out[OutPath.WR0_HI] = OutSel.ALU_OUT
    u.out_enable[OutPath.WR0_LO] = 1
    u.out_enable[OutPath.WR0_HI] = 1
    return [u]


_dummy1 = Spec(body=Src0 * C0 + Src1 * C1,
               reference=lambda in0, in1, s0, s1, imm2: in0)
_dummy2 = Spec(body=Src0 * C0 + Src1 * C1,
               reference=lambda in0, in1, s0, s1, imm2: in0)

FIR2S = _register(_HandOp("FIR2S_ANT", _fir2s_1x, _fir2s_2x, True, _dummy1))
FIR2ADDS = _register(
    _HandOp("FIR2ADDS_ANT", _fir2adds_1x, _fir2adds_2x, True, _dummy2)
)


def _emit_dve(eng, op, *, out, in0, in1, s0, s1, perf_max):
    """Copy of bass.Vector._custom_dve trimmed to the TTSS shape, plus
    perf_max (the stock emitter writes perf mode Disable unconditionally)."""
    nc = eng.bass
    if op.name not in nc.m.ant_custom_dve_ops:
        nc.m.ant_custom_dve_ops = sorted({*nc.m.ant_custom_dve_ops, op.name})
    ver = "v3"
    compiled = op.compile(ver)
    shape = bass_isa.CustomDveShape.TTSS
    isa_opcode = nc.isa.Opcode[
        f"NEURON_ISA_TPB_OPCODE_CUSTOM_DVE_ANT_{shape.slot()}"
    ].value
    ins = [eng.lower_ap(in0, for_isa=True, opt=True),
           eng.lower_ap(in1, for_isa=True, opt=True),
           eng.lower_ap(s0, for_isa=True),
           eng.lower_ap(s1, for_isa=True)]
    outs = [eng.lower_ap(out, for_isa=True, opt=True)]
    return eng.add_instruction(
        bass_isa.InstCustomDveAnt(
            name=nc.get_next_instruction_name(),
            op_name=op.name,
            rd1_en=True,
            subdim=0,
            imm2=0.0,
            perf_max=perf_max,
            shape=shape,
            row=dve_ops.get_dve_sub_opcode(op.name),
            isa_opcode=isa_opcode,
            ins=ins,
            outs=outs,
        )
    )


# --- kernel ---------------------------------------------------------------- #

def _build_nc():
    nc = bacc.Bacc(
        "TRN2",
        target_bir_lowering=False,
        debug=False,
        enable_asserts=False,
        num_devices=N_CORES,
    )
    x = nc.dram_tensor("x", [R, SP], BF16, kind="ExternalInput").ap()
    # host-prearranged: w[p, g*K+k] = weight for row (g*128+p), tap k
    w = nc.dram_tensor("w", [P, N_GROUPS * K], F32, kind="ExternalInput").ap()
    y = nc.dram_tensor("y", [R, S], BF16, kind="ExternalOutput").ap()

    def chunks_for_group(g):
        # Taper both ends: small first chunks so compute starts as soon as
        # ~516 cols land, and a small final chunk so the last store (which
        # serializes behind the last DVE op) is only 256KB.
        if g == 0:
            return [(0, 256), (256, 768), (1024, 1536), (2560, 1536)]
        if g == N_GROUPS - 1:
            return [(0, 1536), (1536, 1536), (3072, 512), (3584, 512)]
        return [(0, 4096)]

    with tile.TileContext(nc) as tc:
        with ExitStack() as ctx:
            # bufs sized so 3-4 x loads stay in flight ahead of the DVE —
            # at bufs=4 the buffer recycle exposed ~3us of transfer latency
            # every 4th chunk.
            x_pool = ctx.enter_context(tc.tile_pool(name="x", bufs=5))
            const_pool = ctx.enter_context(tc.tile_pool(name="const", bufs=1))
            p_pool = ctx.enter_context(tc.tile_pool(name="p", bufs=2))
            out_pool = ctx.enter_context(tc.tile_pool(name="out", bufs=5))

            # w is tiny (16KB) — issue it first so it completes while the
            # first x chunk streams; both gate the first FIR2S.
            w_all = const_pool.tile([P, N_GROUPS * K], F32)
            nc.sync.dma_start(w_all[:], w[:])
            xt0 = x_pool.tile([P, 256 + 4], BF16, tag="x")
            nc.sync.dma_start(xt0[:], x[0:P, 0 : 256 + 4])
            w_all3 = w_all[:].rearrange("p (g k) -> p g k", g=N_GROUPS)

            for g in range(N_GROUPS):
                rows = slice(g * P, (g + 1) * P)
                wt = w_all3[:, g, :]
                for off, tl in chunks_for_group(g):
                    n = tl + 2
                    if g == 0 and off == 0:
                        xt = xt0
                    else:
                        xt = x_pool.tile([P, tl + 4], BF16, tag="x")
                        nc.sync.dma_start(xt[:], x[rows, off : off + tl + 4])

                    # p[j] = w3*xt[j+2] + w2*xt[j+1]; cols 0,1 are lead-in
                    p = p_pool.tile([P, n], BF16, tag="p")
                    _emit_dve(
                        nc.vector, FIR2S, out=p[:],
                        in0=xt[:, 2 : 2 + n], in1=xt[:, 0:n],
                        s0=wt[:, 3:4], s1=wt[:, 2:3], perf_max=PERF_MAX,
                    )
                    # y[j] = w1*xt[j] + w0*xt[j-1] + p[j]
                    ye = out_pool.tile([P, n], BF16, tag="ye")
                    _emit_dve(
                        nc.vector, FIR2ADDS, out=ye[:],
                        in0=xt[:, 0:n], in1=p[:],
                        s0=wt[:, 1:2], s1=wt[:, 0:1], perf_max=PERF_MAX,
                    )
                    # out-DMAs ride the ACT HWDGE queue so a stalled output
                    # never head-of-line-blocks the next x-tile load; split
                    # big stores in halves to keep the output stream smooth
                    if tl > 2048:
                        hl = tl // 2
                        nc.scalar.dma_start(
                            y[rows, off : off + hl], ye[:, 2 : 2 + hl]
                        )
                        nc.scalar.dma_start(
                            y[rows, off + hl : off + tl],
                            ye[:, 2 + hl : 2 + tl],
                        )
                    else:
                        nc.scalar.dma_start(
                            y[rows, off : off + tl], ye[:, 2 : 2 + tl]
                        )
    nc.compile()
    return nc


_NC_CACHE = None


def _get_nc():
    global _NC_CACHE
    if _NC_CACHE is None:
        _NC_CACHE = _build_nc()
    return _NC_CACHE


def _run(in_maps, trace=False, **kwargs):
    nc = _get_nc()
    return bass_utils.run_bass_kernel_spmd(
        nc, in_maps, core_ids=list(range(N_CORES)), trace=trace, **kwargs
    )


def _prepare_in_maps(hidden_states, weight):
    x = np.asarray(hidden_states, dtype=np.float32).astype(BF)
    w = np.asarray(weight, dtype=np.float32)
    # Channel-major, zero-padded: xt[h, b, PAD+s] = x[b, s, h]
    xt = np.zeros((H, B, SP), dtype=BF)
    xt[:, :, PAD:] = x.transpose(2, 0, 1)
    xt = xt.reshape(N_CORES, R, SP)
    # w_prep[core][p, g*K+k] = weight for row (g*128+p) of that core
    w_rows = np.repeat(w, B, axis=0).reshape(N_CORES, N_GROUPS, P, K)
    w_prep = np.ascontiguousarray(
        w_rows.transpose(0, 2, 1, 3).reshape(N_CORES, P, N_GROUPS * K)
    )
    return [{"x": xt[k], "w": w_prep[k]} for k in range(N_CORES)]


def _assemble(results):
    yt = np.empty((H, B, S), dtype=np.float32)
    for k in range(N_CORES):
        yk = np.asarray(results[k]["y"]).view(BF) if results[k]["y"].dtype != BF \
            else results[k]["y"]
        yt[k * C : (k + 1) * C] = yk.astype(np.float32).reshape(C, B, S)
    return np.ascontiguousarray(yt.transpose(1, 2, 0))


def kernel(hidden_states, weight, attention_mask_2d):
    assert hidden_states.shape == (B, S, H)
    assert weight.shape == (H, K)
    in_maps = _prepare_in_maps(hidden_states, weight)
    res = _run(in_maps)
    y = _assemble(res.results)
    mask = np.asarray(attention_mask_2d, dtype=np.float32)
    if not np.all(mask == 1.0):
        y = y * mask[:, :, None]
    return y


def kernel_traced(hidden_states, weight, attention_mask_2d, **kwargs):
    """Same as kernel() but returns (y, BassKernelResults) with profiling."""
    in_maps = _prepare_in_maps(hidden_states, weight)
    res = _run(in_maps, trace=True, **kwargs)
    y = _assemble(res.results)
    mask = np.asarray(attention_mask_2d, dtype=np.float32)
    if not np.all(mask == 1.0):
        y = y * mask[:, :, None]
    return y, res
